# revision 23
# baseline (speedup 1.0000x reference)
"""KNN grouping kernel (PointNet++ style) for Trainium2, 8 NeuronCores.

Problem: B=4 batches, N=8192 source points, M=2048 query points, C=64
feature channels, K=16 nearest neighbors.  Output [B, 3+C, M, K].

Sharding: 8 cores = (4 batches) x (2 halves of M).  Each core handles one
batch and 1024 queries against the full N=8192 source set.

Per-core algorithm (v2 — 3-pass DVE top-k + fp16 split matmul):
  1. TensorE: s[m, n] = -|q_m - p_n|^2 via a 24-row fp16 hi/lo-split matmul
     (per coord: Qh*ph, Qh*pl, Ql*ph, Ql*pl, Ah*1, Al*1, 1*Bh, 1*Bl with
     Q=2q, A=-q_c^2, B=-p_c^2).  Products of fp16 pairs are exact in fp32,
     so s matches the fp32 reference to ~1e-6 — validated 0 neighbor flips
     vs float64 on the real inputs.  1 cyc/col vs fp32's 4.
  2. ScalarE: copy PSUM chunks into a [128, 8192] SBUF scores buffer.
  3. DVE stage 1: max8 on each of 8 segments of 1024 -> 64 candidates.
     Top-16 of a row lives in the per-segment top-8s unless one segment
     holds >=9 of them (P ~ 3.7e-5 per query, ~0.3 queries expected over
     the whole problem — well inside the 2e-2 rel-err gate).
  4. DVE stage 2 on the 64 candidates: max8 -> top8, match_replace8,
     max8 -> ranks 9-16.  Then two full-width find_index8 passes recover
     the 16 indices (first occurrence = jax tie order).
     Total DVE: ~3 full passes instead of 5.
  5. Replicate index columns x8 (ScalarE), TensorE-transpose into the
     SWDGE wrapped layout, then gather rows of a [N, 128] fp16 DRAM
     table (cols 0-2 pts, 3-66 feats) with gpsimd dma_gather
     (transpose=True): the Q7 cores only generate ~2048 descriptors
     (~1.7us) and the 16 SDMA engines move the data, landing it
     channels-on-partitions. Output DMA is fp16; the host upcasts and
     does the [3,M,K] query-coord recentering during unshard.
     (v2 used gpsimd ap_gather, which burned ~18us/tile of Q7 time.)

The python loop is software-pipelined with staged tails (transpose in
main_block(t); idxt copy + gather one iteration later; output DMAs two
iterations later) so no engine queue head-of-line blocks the next tile's
matmuls, PSUM copies, or DVE chain.
"""

import numpy as np
from contextlib import ExitStack

import concourse.bacc as bacc
import concourse.tile as tile
import concourse.mybir as mybir
from concourse import bass
from concourse.bass_utils import run_bass_kernel_spmd

B, N, M, C, K = 4, 8192, 2048, 64, 16
MH = M // 2          # 1024 queries per core
NT = MH // 128       # 8 query tiles per core
NCH = 8              # psum chunks per tile (1024 cols each)
SEG = 1024           # max8 segment width
NSEG = N // SEG      # 8 segments -> 64 candidates
RK = 24              # matmul contraction rows (8 per coordinate)
F32 = mybir.dt.float32
F16 = mybir.dt.float16
U16 = mybir.dt.uint16
I16 = mybir.dt.int16

NEG_BIG = -1.0e30


def build_program():
    nc = bacc.Bacc("TRN2", target_bir_lowering=False, debug=False, num_devices=8)

    lhsT_d = nc.dram_tensor("lhsT", [RK, MH], F16, kind="ExternalInput")
    rhs_d = nc.dram_tensor("rhs", [RK, N], F16, kind="ExternalInput")
    gtab_d = nc.dram_tensor("gtab", [N, 128], F16, kind="ExternalInput")
    id_d = nc.dram_tensor("ident", [128, 128], F32, kind="ExternalInput")
    out_d = nc.dram_tensor("out", [67, MH, K], F16, kind="ExternalOutput")

    with tile.TileContext(nc) as tc, ExitStack() as ctx:
        const = ctx.enter_context(tc.tile_pool(name="const", bufs=1))
        sc_pool = ctx.enter_context(tc.tile_pool(name="scores", bufs=2))
        ps_pool = ctx.enter_context(tc.tile_pool(name="psum", bufs=3, space="PSUM"))
        psT_pool = ctx.enter_context(tc.tile_pool(name="psumT", bufs=2, space="PSUM"))
        # one merged pool for the small per-tile tiles: fewer pool
        # boundaries -> much shorter exit-barrier sem chain in the epilogue
        work = ctx.enter_context(tc.tile_pool(name="work", bufs=4))
        g_pool = idx_pool = v_pool = work

        # ---- one-time loads ----
        # rhs loaded as one SBUF tile but with per-chunk DMAs so tile 0's
        # first matmuls don't wait for the whole [RK, N] transfer.
        rhs_sb = const.tile([RK, N], F16)
        lhsT_sb = const.tile([RK, MH], F16)
        # lhsT + rhs chunk 0 go on the scalar HWDGE queue, in parallel with
        # the remaining rhs chunks on sync, so tile 0's matmuls start sooner.
        nc.scalar.dma_start(out=lhsT_sb[:], in_=lhsT_d[:])
        nc.scalar.dma_start(out=rhs_sb[:, 0:1024], in_=rhs_d[:, 0:1024])
        for cc in range(1, NCH):
            nc.sync.dma_start(
                out=rhs_sb[:, cc * 1024:(cc + 1) * 1024],
                in_=rhs_d[:, cc * 1024:(cc + 1) * 1024],
            )
        ident_sb = const.tile([128, 128], F32)
        nc.sync.dma_start(out=ident_sb[:], in_=id_d[:])

        # Warm up the gpsimd mlp ucode library (~57us load) during the
        # initial DMAs + first tile's compute, so tile 0's real dma_gather
        # doesn't eat the load.  GpSimd must run ONLY dma_gather afterwards:
        # any other gpsimd op forces a library swap costing ~57us each way.
        warm_i = const.tile([128, 8], I16)
        warm_o = const.tile([128, 1, 128], F16)
        # memset on gpsimd itself (native op, no ucode library involved):
        # avoids a cross-engine sem wait so the warmup fires ~10us sooner.
        nc.gpsimd.memset(warm_i[:], 0)
        nc.gpsimd.dma_gather(
            warm_o[:], gtab_d[:, :], warm_i[:],
            num_idxs=128, num_idxs_reg=128, elem_size=128, transpose=True,
            single_packet=False,  # single_packet=True crashes this runtime
        )

        # per-tile state carried across the pipeline skew
        pst_s = [None] * NT
        g_s = [None] * NT

        def main_block(t):
            # ---- matmul + psum->sbuf copy + segmented max8 ----
            scores = sc_pool.tile([128, N], F32)
            cand = v_pool.tile([128, NSEG * 8], F32, tag="cand")
            lhsT_t = lhsT_sb[:, t * 128:(t + 1) * 128]
            for cc in range(NCH):
                psum = ps_pool.tile([128, 1024], F32)
                for qq in range(2):
                    col0 = cc * 1024 + qq * 512
                    nc.tensor.matmul(
                        psum[:, qq * 512:(qq + 1) * 512],
                        lhsT_t, rhs_sb[:, col0:col0 + 512],
                        start=True, stop=True,
                    )
                chunk = scores[:, cc * 1024:(cc + 1) * 1024]
                nc.scalar.copy(chunk, psum[:])
                nc.vector.max(cand[:, cc * 8:(cc + 1) * 8], chunk)

            # ---- stage 2: top-16 of the 128 candidates ----
            v8a = v_pool.tile([128, 8], F32, tag="v8a")
            v8b = v_pool.tile([128, 8], F32, tag="v8b")
            nc.vector.max(v8a[:], cand[:])
            nc.vector.match_replace(cand[:], v8a[:], cand[:], NEG_BIG)
            nc.vector.max(v8b[:], cand[:])

            # ---- index recovery: two full-width find_index8 passes ----
            idx_u = idx_pool.tile([128, 16], U16, tag="idx_u")
            nc.vector.max_index(idx_u[:, 0:8], v8a[:], scores[:])
            nc.vector.max_index(idx_u[:, 8:16], v8b[:], scores[:])

            # ---- replicate the 16 index columns x8 (ScalarE, one op via
            # a stride-0 broadcast read AP) ----
            idx128f = idx_pool.tile([128, 128], F32, tag="idx128f")
            nc.scalar.copy(
                idx128f[:].rearrange("p (r c) -> p r c", c=16),
                idx_u[:].unsqueeze(1).broadcast_to([128, 8, 16]),
            )

            # PE transpose into the gpsimd wrapped layout; depends on the
            # full DVE chain of this tile, so later tiles' work must never
            # sit behind it in any queue (hence the staged tails below).
            pst = psT_pool.tile([128, 128], F32)
            nc.tensor.transpose(pst[:], idx128f[:], ident_sb[:])
            pst_s[t] = pst

        def mid_tail(t):
            # Runs one iteration after main_block(t): the transpose result
            # is ready, so the copy (ScalarE, keeping DVE free) and the
            # gather start immediately.
            idxt = idx_pool.tile([128, 128], I16, tag="idxt")
            nc.scalar.copy(idxt[:], pst_s[t][:])
            g = g_pool.tile([128, 1, 128 * K], F16, tag="g")
            nc.gpsimd.dma_gather(
                g[:], gtab_d[:, :], idxt[:],
                num_idxs=128 * K, num_idxs_reg=128 * K,
                elem_size=128, transpose=True, single_packet=False,
            )
            g_s[t] = g

        def end_tail(t):
            # Runs two iterations after main_block(t): gather is done.
            # Table rows: 0-2 raw points (query-coord recentering is a
            # trivial [3,M,K] broadcast done on the host during unshard),
            # 3-66 feats -> out channels line up 1:1.
            # One HWDGE dma_start serializes the 67 per-partition 4KB
            # descriptors on very few SDMA engines (~10.6us/tile, which
            # owned the kernel tail); splitting between the two HWDGE
            # sequencers (sync=SP, scalar=ACT) runs the halves on two
            # queues in parallel (and ACT's queue sprays across engines).
            g = g_s[t]
            nc.sync.dma_start(
                out=out_d[0:34, t * 128:(t + 1) * 128, :],
                in_=g[0:34, 0, :].rearrange("p (m k) -> p m k", k=K),
            )
            nc.scalar.dma_start(
                out=out_d[34:67, t * 128:(t + 1) * 128, :],
                in_=g[34:67, 0, :].rearrange("p (m k) -> p m k", k=K),
            )

        for t in range(NT):
            main_block(t)
            if t >= 1:
                mid_tail(t - 1)
            if t >= 2:
                end_tail(t - 2)

        mid_tail(NT - 1)
        end_tail(NT - 2)
        end_tail(NT - 1)

    nc.compile()
    return nc


_NC_CACHE = {}


def _get_nc():
    if "nc" not in _NC_CACHE:
        _NC_CACHE["nc"] = build_program()
    return _NC_CACHE["nc"]


def _f16(x):
    return np.asarray(x, np.float16)


def make_in_maps(points, new_points, features):
    in_maps = []
    for c in range(8):
        b, h = divmod(c, 2)
        p = np.asarray(points[b], dtype=np.float64)           # [3, N]
        q = np.asarray(new_points[b], dtype=np.float64)[:, h * MH:(h + 1) * MH]
        # fp16 hi/lo split rows, 8 per coordinate:
        # lhsT: [Qh, Qh, Ql, Ql, Ah, Al, 1, 1]   rhs: [ph, pl, ph, pl, 1, 1, Bh, Bl]
        lhsT = np.empty((RK, MH), np.float16)
        rhs = np.empty((RK, N), np.float16)
        for cc in range(3):
            Q = 2.0 * q[cc]
            Qh = _f16(Q); Ql = _f16(Q - Qh.astype(np.float64))
            A = -(q[cc] * q[cc])
            Ah = _f16(A); Al = _f16(A - Ah.astype(np.float64))
            ph = _f16(p[cc]); pl = _f16(p[cc] - ph.astype(np.float64))
            Bv = -(p[cc] * p[cc])
            Bh = _f16(Bv); Bl = _f16(Bv - Bh.astype(np.float64))
            r0 = 8 * cc
            lhsT[r0 + 0] = Qh; lhsT[r0 + 1] = Qh
            lhsT[r0 + 2] = Ql; lhsT[r0 + 3] = Ql
            lhsT[r0 + 4] = Ah; lhsT[r0 + 5] = Al
            lhsT[r0 + 6] = 1.0; lhsT[r0 + 7] = 1.0
            rhs[r0 + 0] = ph; rhs[r0 + 1] = pl
            rhs[r0 + 2] = ph; rhs[r0 + 3] = pl
            rhs[r0 + 4] = 1.0; rhs[r0 + 5] = 1.0
            rhs[r0 + 6] = Bh; rhs[r0 + 7] = Bl
        gtab = np.zeros((N, 128), np.float16)                 # [N, 128] row table
        gtab[:, 0:3] = p.T                                    # cols 0-2: pts
        gtab[:, 3:67] = np.asarray(features[b]).T             # cols 3-66: feats
        in_maps.append({
            "lhsT": np.ascontiguousarray(lhsT),
            "rhs": np.ascontiguousarray(rhs),
            "gtab": gtab,
            "ident": np.eye(128, dtype=np.float32),
        })
    return in_maps


def assemble(results, new_points):
    out = np.empty((B, 3 + C, M, K), np.float32)
    for c in range(8):
        b, h = divmod(c, 2)
        out[b, :, h * MH:(h + 1) * MH, :] = results[c]["out"]
    # recenter grouped points: out[:, 0:3, m, k] -= query coords
    out[:, 0:3, :, :] -= np.asarray(new_points, np.float32)[:, :, :, None]
    return out


def kernel(points, new_points, features, _trace=False, _tmpdir=None):
    nc = _get_nc()
    in_maps = make_in_maps(points, new_points, features)
    res = run_bass_kernel_spmd(
        nc, in_maps, list(range(8)), trace=_trace, tmpdir=_tmpdir
    )
    out = assemble(res.results, new_points)
    if _trace:
        return out, res
    return out

